# revision 1
# baseline (speedup 1.0000x reference)
"""Trainium2 Bass kernel for nn_LocationAwareMSAGAT_Net.

Strategy: data-parallel over batch B=8 across the 8 NeuronCores (one batch
element per core); all parameters replicated.  Per core:

  phase A: multi-scale dilated conv (as 24 shifted matmuls, bf16) + BN fold
           + SiLU (ScalarE, conv bias folded into activation bias)
  phase B: bottleneck (alpha folded into W_low; accumulated in PSUM over
           scales) -> W_high -> +residual -> LayerNorm1 -> transpose (PE)
  phase C: GAT projections: one matmul computes Wh for all heads plus
           src/dst attention logits (gat_W@a_src / gat_W@a_dst appended as
           extra columns)
  phase D: attention, computed transposed (P^T[m,q] tiles):
           C = src[q] + dst[m] + maskbias  (GpSimd, additive -1e9 mask)
           leaky-relu via one fused DVE op  max(x, 0.2x)
           exp on ScalarE (batched [128, 8192] per head)
           hp^T = [Wh_h | ones]^T @ P^T accumulated in PSUM over m-chunks
           (ones column yields softmax denominators)
           PE-transpose back, divide rows by denominator
  phase E: LayerNorm2 -> DMA out

Everything on the PE is bf16 with fp32 PSUM accumulation.
"""

import os
import numpy as np
import ml_dtypes
from contextlib import ExitStack

import concourse.bass as bass
import concourse.tile as tile
from concourse import bacc, mybir
from concourse.bass_utils import run_bass_kernel_spmd
from concourse.masks import make_identity

BF = mybir.dt.bfloat16
F32 = mybir.dt.float32
EPS = 1e-5
NEG = -1e9

B, N, H = 8, 1024, 256
S, K, HEADS = 4, 3, 4
D = H // HEADS          # 64
NCH = N // 128          # 8 chunks of 128
CCH = H // 128          # 2 channel chunks
BOT = 8                 # bottleneck dim

_CACHED = {}


def _build(trivial: dict) -> bass.Bass:
    nc = bacc.Bacc("TRN2", target_bir_lowering=False, debug=False,
                   num_devices=B)

    xt_d = nc.declare_dram_parameter("xt", [CCH, 128, N], BF, isOutput=False)
    xres_d = nc.declare_dram_parameter("xres", [N, H], F32, isOutput=False)
    wt_d = nc.declare_dram_parameter("wt", [S * K * CCH, 128, H], BF, isOutput=False)
    bconv_d = nc.declare_dram_parameter("bconv", [128, S * CCH], F32, isOutput=False)
    wlow_d = nc.declare_dram_parameter("wlow", [S * CCH, 128, BOT], BF, isOutput=False)
    whigh_d = nc.declare_dram_parameter("whigh", [BOT, H], BF, isOutput=False)
    g_d = nc.declare_dram_parameter("gmat", [CCH, 128, H + 2 * HEADS], BF,
                                    isOutput=False)
    mask_d = nc.declare_dram_parameter("maskT", [NCH, 128, N], BF, isOutput=False)
    wsr_d = nc.declare_dram_parameter("wsrcrep", [HEADS, CCH, 128, 128], BF,
                                      isOutput=False)
    out_d = nc.declare_dram_parameter("out", [N, H], F32, isOutput=True)

    with tile.TileContext(nc) as tc:
        with ExitStack() as ctx:
            _body(ctx, tc, xt_d, xres_d, wt_d, bconv_d, wlow_d, whigh_d, g_d,
                  mask_d, wsr_d, out_d)
    nc.compile()
    return nc


def _body(ctx, tc, xt_d, xres_d, wt_d, bconv_d, wlow_d, whigh_d, g_d,
          mask_d, wsr_d, out_d):
    nc = tc.nc
    consts = ctx.enter_context(tc.tile_pool(name="consts", bufs=1))
    work = ctx.enter_context(tc.tile_pool(name="work", bufs=3))
    statp = ctx.enter_context(tc.tile_pool(name="stats", bufs=4))
    outp = ctx.enter_context(tc.tile_pool(name="outp", bufs=3))
    ptp = ctx.enter_context(tc.tile_pool(name="ptp", bufs=2))
    ptlp = ctx.enter_context(tc.tile_pool(name="ptlp", bufs=2))
    cp = ctx.enter_context(tc.tile_pool(name="cp", bufs=3))

    # ---------------- constants / inputs into SBUF ----------------
    xpad = consts.tile([128, CCH, N + 16], BF, tag="xpad")
    nc.vector.memset(xpad[:, :, 0:8], 0.0)
    nc.vector.memset(xpad[:, :, N + 8:N + 16], 0.0)
    nc.sync.dma_start(out=xpad[:, :, 8:8 + N],
                      in_=xt_d.rearrange("c p n -> p c n"))

    wt_sb = consts.tile([128, S * K * CCH, H], BF, tag="wt")
    nc.sync.dma_start(out=wt_sb[:], in_=wt_d.rearrange("t p h -> p t h"))

    bconv_sb = consts.tile([128, S * CCH], F32, tag="bconv")
    nc.sync.dma_start(out=bconv_sb[:], in_=bconv_d[:])

    wlow_sb = consts.tile([128, S * CCH, BOT], BF, tag="wlow")
    nc.sync.dma_start(out=wlow_sb[:], in_=wlow_d.rearrange("t p b -> p t b"))

    whigh_sb = consts.tile([BOT, H], BF, tag="whigh")
    nc.sync.dma_start(out=whigh_sb[:], in_=whigh_d[:])

    g_sb = consts.tile([128, CCH, H + 2 * HEADS], BF, tag="gmat")
    nc.sync.dma_start(out=g_sb[:], in_=g_d.rearrange("c p h -> p c h"))

    mask_sb = consts.tile([128, NCH, N], BF, tag="mask")
    nc.sync.dma_start(out=mask_sb[:], in_=mask_d.rearrange("j p n -> p j n"))

    wsr_sb = consts.tile([128, HEADS, CCH, 128], BF, tag="wsr")
    nc.sync.dma_start(out=wsr_sb[:],
                      in_=wsr_d.rearrange("h c p x -> p h c x"))

    xres_sb = consts.tile([128, NCH, H], F32, tag="xres")
    nc.sync.dma_start(
        out=xres_sb[:],
        in_=xres_d.rearrange("(nch p) h -> p nch h", p=128))

    ident_bf = consts.tile([128, 128], BF, tag="idbf")
    make_identity(nc, ident_bf[:])
    ident_f32 = consts.tile([128, 128], F32, tag="idf32")
    make_identity(nc, ident_f32[:])
    eps_sb = consts.tile([128, 1], F32, tag="eps")
    nc.vector.memset(eps_sb[:], EPS)
    zero_sb = consts.tile([128, 1], F32, tag="zero")
    nc.vector.memset(zero_sb[:], 0.0)

    # persistent intermediates
    fused_sb = consts.tile([128, S, CCH, N], BF, tag="fused")
    lowT_sb = consts.tile([BOT, N], BF, tag="lowT")
    h_all = consts.tile([128, NCH, H], F32, tag="h_all")
    mv1 = consts.tile([128, NCH, 2], F32, tag="mv1")
    rstd1 = consts.tile([128, NCH], F32, tag="rstd1")
    hT_sb = consts.tile([128, CCH, N], BF, tag="hT")
    wh_all = consts.tile([128, NCH, HEADS * (D + 1)], BF, tag="wh")
    nc.vector.memset(
        wh_all[:].rearrange("p j (h x) -> p j h x", x=D + 1)[:, :, :, D], 1.0)
    sd_sb = consts.tile([128, NCH, 2 * HEADS], F32, tag="sd")
    srcb_sb = consts.tile([128, HEADS, N], BF, tag="srcb")
    hp_all = consts.tile([128, NCH, H], F32, tag="hp")
    mv2 = consts.tile([128, NCH, 2], F32, tag="mv2")
    rstd2 = consts.tile([128, NCH], F32, tag="rstd2")

    # ---------------- phase A: conv + silu ----------------
    ctxA = ExitStack()
    convp = ctxA.enter_context(tc.tile_pool(name="convp", bufs=3, space="PSUM"))
    lowp = ctxA.enter_context(tc.tile_pool(name="lowp", bufs=2, space="PSUM"))
    for cout in range(CCH):
        for nch in range(2):           # halves of N, 512 wide
            for i in range(S):
                ps = convp.tile([128, 512], F32, tag="conv")
                dil = 2 ** i
                first = True
                for c in range(CCH):
                    for k in range(K):
                        sh = (k - 1) * dil
                        t = (i * K + k) * CCH + c
                        nc.tensor.matmul(
                            ps[:],
                            lhsT=wt_sb[:, t, cout * 128:(cout + 1) * 128],
                            rhs=xpad[:, c, 8 + sh + nch * 512:
                                     8 + sh + nch * 512 + 512],
                            start=first, stop=(c == CCH - 1 and k == K - 1))
                        first = False
                dst = fused_sb[:, i, cout, nch * 512:nch * 512 + 512]
                bias_ap = bconv_sb[:, i * CCH + cout:i * CCH + cout + 1]
                if os.environ.get("BASS_SIM_COMPAT", "0") == "1":
                    # CoreSim has no Silu: sigmoid + fused (ps+b)*sig on DVE
                    sg = work.tile([128, 512], F32, tag="sg")
                    nc.scalar.activation(
                        out=sg[:], in_=ps[:],
                        func=mybir.ActivationFunctionType.Sigmoid,
                        bias=bias_ap, scale=1.0)
                    nc.vector.scalar_tensor_tensor(
                        out=dst, in0=ps[:], scalar=bias_ap, in1=sg[:],
                        op0=mybir.AluOpType.add, op1=mybir.AluOpType.mult)
                else:
                    nc.scalar.activation(
                        out=dst, in_=ps[:],
                        func=mybir.ActivationFunctionType.Silu,
                        bias=bias_ap, scale=1.0)

    # ---------------- phase A2: lowT = sum_i (a_i W_low)^T @ silu_i --------
    for nch in range(2):
        lps = lowp.tile([BOT, 512], F32, tag="low")
        first = True
        for i in range(S):
            for c in range(CCH):
                nc.tensor.matmul(
                    lps[:],
                    lhsT=wlow_sb[:, i * CCH + c, :],
                    rhs=fused_sb[:, i, c, nch * 512:nch * 512 + 512],
                    start=first, stop=(i == S - 1 and c == CCH - 1))
                first = False
        nc.vector.tensor_copy(out=lowT_sb[:, nch * 512:nch * 512 + 512],
                              in_=lps[:])
    ctxA.close()

    # ---------------- phase B: high + residual + ln1 + transpose ----------
    ctxB = ExitStack()
    psA = ctxB.enter_context(tc.tile_pool(name="psB", bufs=2, space="PSUM"))
    psTr = ctxB.enter_context(tc.tile_pool(name="psTrB", bufs=3, space="PSUM"))
    for q in range(NCH):
        hps = psA.tile([128, H], F32, tag="high")
        nc.tensor.matmul(hps[:], lhsT=lowT_sb[:, q * 128:(q + 1) * 128],
                         rhs=whigh_sb[:], start=True, stop=True)
        nc.vector.tensor_add(h_all[:, q, :], hps[:], xres_sb[:, q, :])
        st = statp.tile([128, 6], F32, tag="bn1")
        nc.vector.bn_stats(out=st[:], in_=h_all[:, q, :])
        nc.vector.bn_aggr(out=mv1[:, q, :], in_=st[:])

    # rstd1 = exp(-0.5 * ln(var + eps))  (one table set: natural_log+exp)
    nc.scalar.activation(out=rstd1[:], in_=mv1[:, :, 1],
                         func=mybir.ActivationFunctionType.Ln, bias=eps_sb[:],
                         scale=1.0)
    nc.scalar.activation(out=rstd1[:], in_=rstd1[:],
                         func=mybir.ActivationFunctionType.Exp, bias=zero_sb[:],
                         scale=-0.5)

    for q in range(NCH):
        hn = work.tile([128, H], BF, tag="hn")
        nc.vector.tensor_scalar(
            out=hn[:], in0=h_all[:, q, :],
            scalar1=mv1[:, q, 0:1], scalar2=rstd1[:, q:q + 1],
            op0=mybir.AluOpType.subtract, op1=mybir.AluOpType.mult)
        for c in range(CCH):
            tp = psTr.tile([128, 128], BF, tag="trh")
            nc.tensor.transpose(out=tp[:], in_=hn[:, c * 128:(c + 1) * 128],
                                identity=ident_bf[:])
            nc.vector.tensor_copy(out=hT_sb[:, c, q * 128:(q + 1) * 128],
                                  in_=tp[:])

    ctxB.close()
    # ---------------- phase C: GAT projections ----------------
    ctxC = ExitStack()
    psA = ctxC.enter_context(tc.tile_pool(name="psC", bufs=2, space="PSUM"))
    psTr = ctxC.enter_context(tc.tile_pool(name="psTrC", bufs=2, space="PSUM"))
    for j in range(NCH):
        gps = psA.tile([128, H + 2 * HEADS], F32, tag="gat")
        for c in range(CCH):
            nc.tensor.matmul(gps[:], lhsT=hT_sb[:, c, j * 128:(j + 1) * 128],
                             rhs=g_sb[:, c, :], start=(c == 0),
                             stop=(c == CCH - 1))
        whj = wh_all[:, j, :].rearrange("p (h x) -> p h x", x=D + 1)
        nc.scalar.copy(
            out=whj[:, :, 0:D],
            in_=gps[:, 0:H].rearrange("p (h x) -> p h x", x=D))
        nc.vector.tensor_copy(out=sd_sb[:, j, :], in_=gps[:, H:H + 2 * HEADS])

    # src_bcast[h][p, q] = src_h[q] for all p, via replicated-column matmul
    for h in range(HEADS):
        for half in range(2):
            sps = psTr.tile([128, 512], F32, tag="sbc")
            for c in range(CCH):
                nc.tensor.matmul(
                    sps[:], lhsT=wsr_sb[:, h, c, :],
                    rhs=hT_sb[:, c, half * 512:half * 512 + 512],
                    start=(c == 0), stop=(c == CCH - 1))
            nc.scalar.copy(out=srcb_sb[:, h, half * 512:half * 512 + 512],
                           in_=sps[:])

    ctxC.close()
    # ---------------- phase D: attention ----------------
    ctxD = ExitStack()
    attp = ctxD.enter_context(tc.tile_pool(name="attp", bufs=4, space="PSUM"))
    psTr = ctxD.enter_context(tc.tile_pool(name="psTrD", bufs=3, space="PSUM"))
    for h in range(HEADS):
        ptl = ptlp.tile([128, NCH, N], BF, tag="ptl")
        pt = ptp.tile([128, NCH, N], BF, tag="pt")
        for j in range(NCH):
            cs = cp.tile([128, N], BF, tag="cs")
            nc.gpsimd.tensor_tensor(
                out=cs[:], in0=srcb_sb[:, h, :], in1=mask_sb[:, j, :],
                op=mybir.AluOpType.add)
            ct = cp.tile([128, N], BF, tag="ct")
            nc.vector.tensor_scalar_add(
                out=ct[:], in0=cs[:],
                scalar1=sd_sb[:, j, HEADS + h:HEADS + h + 1])
            nc.vector.scalar_tensor_tensor(
                out=ptl[:, j, :], in0=ct[:], scalar=0.2, in1=ct[:],
                op0=mybir.AluOpType.mult, op1=mybir.AluOpType.max)
        nc.scalar.activation(out=pt[:], in_=ptl[:],
                             func=mybir.ActivationFunctionType.Exp,
                             bias=zero_sb[:], scale=1.0)

        hp0 = attp.tile([D + 1, 512], F32, tag="hpT")
        hp1 = attp.tile([D + 1, 512], F32, tag="hpT")
        for j in range(NCH):
            for half, hps_ in ((0, hp0), (1, hp1)):
                nc.tensor.matmul(
                    hps_[:],
                    lhsT=wh_all[:, j, h * (D + 1):(h + 1) * (D + 1)],
                    rhs=pt[:, j, half * 512:half * 512 + 512],
                    start=(j == 0), stop=(j == NCH - 1))
        hpt = work.tile([D + 1, N], F32, tag="hpt")
        nc.vector.tensor_copy(out=hpt[:, 0:512], in_=hp0[:])
        nc.vector.tensor_copy(out=hpt[:, 512:N], in_=hp1[:])
        for q in range(NCH):
            tq = psTr.tile([128, D + 1], F32, tag="trq")
            nc.tensor.transpose(out=tq[:], in_=hpt[:, q * 128:(q + 1) * 128],
                                identity=ident_f32[0:D + 1, 0:D + 1])
            rd = statp.tile([128, 1], F32, tag="rd")
            nc.vector.reciprocal(out=rd[:], in_=tq[:, D:D + 1])
            nc.vector.tensor_scalar_mul(
                out=hp_all[:, q, h * D:(h + 1) * D],
                in0=tq[:, 0:D], scalar1=rd[:])

    ctxD.close()
    # ---------------- phase E: ln2 + out ----------------
    for q in range(NCH):
        st = statp.tile([128, 6], F32, tag="bn2")
        nc.vector.bn_stats(out=st[:], in_=hp_all[:, q, :])
        nc.vector.bn_aggr(out=mv2[:, q, :], in_=st[:])
    nc.scalar.activation(out=rstd2[:], in_=mv2[:, :, 1],
                         func=mybir.ActivationFunctionType.Ln, bias=eps_sb[:],
                         scale=1.0)
    nc.scalar.activation(out=rstd2[:], in_=rstd2[:],
                         func=mybir.ActivationFunctionType.Exp, bias=zero_sb[:],
                         scale=-0.5)
    for q in range(NCH):
        ot = outp.tile([128, H], F32, tag="out")
        nc.vector.tensor_scalar(
            out=ot[:], in0=hp_all[:, q, :],
            scalar1=mv2[:, q, 0:1], scalar2=rstd2[:, q:q + 1],
            op0=mybir.AluOpType.subtract, op1=mybir.AluOpType.mult)
        nc.sync.dma_start(out=out_d[q * 128:(q + 1) * 128, :], in_=ot[:])


def _prep(inputs):
    """Host-side parameter folding. Returns per-core input maps."""
    bf16 = ml_dtypes.bfloat16
    f = lambda a: np.ascontiguousarray(np.asarray(a, np.float32))

    x = f(inputs["x"])
    adj = np.asarray(inputs["adj"])
    conv_w = f(inputs["conv_w"]); conv_b = f(inputs["conv_b"])
    bn_g = f(inputs["bn_g"]); bn_b = f(inputs["bn_b"])
    fw = f(inputs["fusion_weight"])
    W_low = f(inputs["W_low"]); b_low = f(inputs["b_low"])
    W_high = f(inputs["W_high"]); b_high = f(inputs["b_high"])
    ln1_g = f(inputs["ln1_g"]); ln1_b = f(inputs["ln1_b"])
    gat_W = f(inputs["gat_W"])
    a_src = f(inputs["a_src"]); a_dst = f(inputs["a_dst"])
    ln2_g = f(inputs["ln2_g"]); ln2_b = f(inputs["ln2_b"])

    trivial = dict(
        b_low=np.allclose(b_low, 0), b_high=np.allclose(b_high, 0),
        ln1=np.allclose(ln1_g, 1) and np.allclose(ln1_b, 0),
        ln2=np.allclose(ln2_g, 1) and np.allclose(ln2_b, 0))
    if not all(trivial.values()):
        raise NotImplementedError(f"non-trivial affine params: {trivial}")

    alpha = np.exp(fw - fw.max()); alpha /= alpha.sum()
    gprime = bn_g / np.float32(np.sqrt(1.0 + EPS))          # [S,H]
    bconv = conv_b * gprime + bn_b                           # [S,H]
    # Wt[i,k,cin,cout] = conv_w[i,cout,cin,k]*gprime[i,cout]
    Wt = np.transpose(conv_w, (0, 3, 2, 1)) * gprime[:, None, None, :]
    # [S,K,cin,H] -> [S,K,CCH,128,H] -> [S*K*CCH,128,H]
    Wt = Wt.reshape(S, K, CCH, 128, H).reshape(S * K * CCH, 128, H)
    # bconv laid out [128, S*CCH]: column i*CCH+c holds channels c*128..c*128+127
    bconv_t = bconv.reshape(S, CCH, 128).transpose(2, 0, 1).reshape(128, S * CCH)

    WlowA = (alpha[:, None, None] * W_low[None]).reshape(S, CCH, 128, BOT)
    WlowA = WlowA.reshape(S * CCH, 128, BOT)

    G = np.zeros((H, H + 2 * HEADS), np.float32)
    for h in range(HEADS):
        G[:, h * D:(h + 1) * D] = gat_W[h]
        G[:, H + h] = gat_W[h] @ a_src[h]
        G[:, H + HEADS + h] = gat_W[h] @ a_dst[h]
    Gr = G.reshape(CCH, 128, H + 2 * HEADS)

    maskT = np.where(adj.T > 0, np.float32(0.0), np.float32(NEG))
    maskTr = maskT.reshape(NCH, 128, N)

    # wsrcrep[h, c, :, j] = (gat_W[h] @ a_src[h])[c*128 + :]  (all 128 cols equal)
    wsrc = np.stack([gat_W[h] @ a_src[h] for h in range(HEADS)])  # [HEADS, H]
    wsrcrep = np.repeat(
        wsrc.reshape(HEADS, CCH, 128, 1), 128, axis=3).astype(np.float32)

    shared = {
        "wt": Wt.astype(bf16),
        "bconv": np.ascontiguousarray(bconv_t),
        "wlow": WlowA.astype(bf16),
        "whigh": W_high.astype(bf16),
        "gmat": Gr.astype(bf16),
        "maskT": maskTr.astype(bf16),
        "wsrcrep": wsrcrep.astype(bf16),
    }
    in_maps = []
    for b in range(B):
        xt = np.ascontiguousarray(x[b].T)                    # [H, N]
        m = dict(shared)
        m["xt"] = xt.reshape(CCH, 128, N).astype(bf16)
        m["xres"] = np.ascontiguousarray(x[b])
        in_maps.append(m)
    return in_maps, trivial


def kernel(**inputs) -> np.ndarray:
    in_maps, trivial = _prep(inputs)
    key = "k"
    if key not in _CACHED:
        _CACHED[key] = _build(trivial)
    nc = _CACHED[key]
    res = run_bass_kernel_spmd(nc, in_maps, list(range(B)))
    out = np.stack([res.results[i]["out"] for i in range(B)], axis=0)
    return out.astype(np.float32)


if __name__ == "__main__":
    import reference
    inputs = {k: np.asarray(v) for k, v in reference.setup_inputs().items()}
    got = kernel(**inputs)
    print("kernel output", got.shape, got.dtype)



# revision 5
# speedup vs baseline: 1.5721x; 1.5721x over previous
"""Trainium2 Bass kernel for nn_LocationAwareMSAGAT_Net.

Strategy: data-parallel over batch B=8 across the 8 NeuronCores (one batch
element per core); all parameters replicated.  Per core:

  phase A: multi-scale dilated conv (as shifted matmuls over full N, bf16)
           + BN fold + SiLU (ScalarE, conv bias folded into activation bias)
  phase B: bottleneck (alpha folded into W_low; accumulated in PSUM over
           scales) -> W_high -> +residual -> LayerNorm1 -> transpose (PE)
  phase C: GAT projections: one matmul computes Wh for all heads plus
           src/dst attention logits (gat_W@a_src / gat_W@a_dst appended as
           extra columns)
  phase D: attention, computed transposed (P^T[m,q] tiles):
           ptl = leaky(srcb[q] + dst[m] + maskbias) in ONE custom DVE op
           (LEAKY_SCORE_ANT: max(y, 0.2y), y = in0 + s0 + in1)
           exp on ScalarE (two [128, 4096] batches per head)
           hp^T = [Wh_h | ones]^T @ P^T accumulated in PSUM over m-chunks
           (ones column yields softmax denominators)
           PE-transpose back, divide rows by denominator
  phase E: LayerNorm2 -> DMA out

Everything on the PE is bf16 with fp32 PSUM accumulation.  PSUM->SBUF
staging copies ride on GpSimd (otherwise idle) to keep DVE free for the
attention elementwise work.
"""

import os
import numpy as np
import ml_dtypes
from contextlib import ExitStack

import concourse.bass as bass
import concourse.tile as tile
from concourse import bacc, mybir
from concourse.bass_utils import run_bass_kernel_spmd
from concourse.masks import make_identity

BF = mybir.dt.bfloat16
F32 = mybir.dt.float32
EPS = 1e-5
NEG = -1e9
LEAKY_SLOPE = 0.2

B, N, H = 8, 1024, 256
S, K, HEADS = 4, 3, 4
D = H // HEADS          # 64
NCH = N // 128          # 8 chunks of 128
CCH = H // 128          # 2 channel chunks
BOT = 8                 # bottleneck dim

_CACHED = {}


def _register_leaky_op():
    """Custom DVE op: out = leaky_relu(in0 + s0 + in1, slope=imm2).

    One DVE pass (1 elem/cycle/lane) replacing the gpsimd-add + DVE-add +
    scalar_tensor_tensor chain.  Registered at runtime into dve_ops.OPS."""
    import concourse.dve_ops as dops
    from concourse.dve_spec import Spec, Src0, Src1, C0, C2, maxx, lower, \
        _has_src1
    from concourse.dve_uop import DveOpSpec

    name = "LEAKY_SCORE_ANT"
    for op in dops.OPS:
        if op.name == name:
            return op

    _y = (Src0 + C0) + Src1

    def _ref(in0, in1, s0, s1, imm2):
        y = (in0.astype(np.float32) + s0) + in1
        return np.maximum(y, y * imm2)

    spec = Spec(body=maxx(_y, _y * C2), reference=_ref)
    row = max(dops._SUB_OPCODE_FOR_NAME.values()) + 1
    dops._SUB_OPCODE_FOR_NAME[name] = row
    uops = lower(spec, ver="v3")
    sha = DveOpSpec(name=name, opcode=row, uops=uops,
                    rd1_en=_has_src1(spec)).sha("v3")
    op = dops.DveOp(name, spec, subdim=False, uops_sha={"v3": sha})
    dops.OPS.append(op)
    dops.CUSTOM_DVE_SPECS[name] = spec
    return op


LEAKY_OP = _register_leaky_op()


def _build(trivial: dict) -> bass.Bass:
    nc = bacc.Bacc("TRN2", target_bir_lowering=False, debug=False,
                   num_devices=B)

    xt_d = nc.declare_dram_parameter("xt", [CCH, 128, N], BF, isOutput=False)
    xres_d = nc.declare_dram_parameter("xres", [N, H], F32, isOutput=False)
    wt_d = nc.declare_dram_parameter("wt", [S * K * CCH, 128, H], BF, isOutput=False)
    bconv_d = nc.declare_dram_parameter("bconv", [128, S * CCH], F32, isOutput=False)
    wlow_d = nc.declare_dram_parameter("wlow", [S * CCH, 128, BOT], BF, isOutput=False)
    whigh_d = nc.declare_dram_parameter("whigh", [BOT, H], BF, isOutput=False)
    g_d = nc.declare_dram_parameter("gmat", [CCH, 128, H + 2 * HEADS], BF,
                                    isOutput=False)
    mask_d = nc.declare_dram_parameter("maskT", [NCH, 128, N], BF, isOutput=False)
    wsr_d = nc.declare_dram_parameter("wsrcrep", [HEADS, CCH, 128, 128], BF,
                                      isOutput=False)
    out_d = nc.declare_dram_parameter("out", [N, H], F32, isOutput=True)

    with tile.TileContext(nc) as tc:
        with ExitStack() as ctx:
            _body(ctx, tc, xt_d, xres_d, wt_d, bconv_d, wlow_d, whigh_d, g_d,
                  mask_d, wsr_d, out_d)
    nc.compile()
    return nc


def _body(ctx, tc, xt_d, xres_d, wt_d, bconv_d, wlow_d, whigh_d, g_d,
          mask_d, wsr_d, out_d):
    nc = tc.nc
    consts = ctx.enter_context(tc.tile_pool(name="consts", bufs=1))
    work = ctx.enter_context(tc.tile_pool(name="work", bufs=3))
    statp = ctx.enter_context(tc.tile_pool(name="stats", bufs=4))
    outp = ctx.enter_context(tc.tile_pool(name="outp", bufs=3))
    ptp = ctx.enter_context(tc.tile_pool(name="ptp", bufs=2))
    ptlp = ctx.enter_context(tc.tile_pool(name="ptlp", bufs=2))

    # ---------------- constants / inputs into SBUF ----------------
    xpad = consts.tile([128, CCH, N + 16], BF, tag="xpad")
    nc.vector.memset(xpad[:, :, 0:8], 0.0)
    nc.vector.memset(xpad[:, :, N + 8:N + 16], 0.0)
    nc.sync.dma_start(out=xpad[:, :, 8:8 + N],
                      in_=xt_d.rearrange("c p n -> p c n"))

    wt_sb = consts.tile([128, S * K * CCH, H], BF, tag="wt")
    nc.sync.dma_start(out=wt_sb[:], in_=wt_d.rearrange("t p h -> p t h"))

    bconv_sb = consts.tile([128, S * CCH], F32, tag="bconv")
    nc.sync.dma_start(out=bconv_sb[:], in_=bconv_d[:])

    wlow_sb = consts.tile([128, S * CCH, BOT], BF, tag="wlow")
    nc.sync.dma_start(out=wlow_sb[:], in_=wlow_d.rearrange("t p b -> p t b"))

    whigh_sb = consts.tile([BOT, H], BF, tag="whigh")
    nc.sync.dma_start(out=whigh_sb[:], in_=whigh_d[:])

    g_sb = consts.tile([128, CCH, H + 2 * HEADS], BF, tag="gmat")
    nc.sync.dma_start(out=g_sb[:], in_=g_d.rearrange("c p h -> p c h"))

    mask_sb = consts.tile([128, NCH, N], BF, tag="mask")
    nc.sync.dma_start(out=mask_sb[:], in_=mask_d.rearrange("j p n -> p j n"))

    wsr_sb = consts.tile([128, HEADS, CCH, 128], BF, tag="wsr")
    nc.sync.dma_start(out=wsr_sb[:],
                      in_=wsr_d.rearrange("h c p x -> p h c x"))

    xres_sb = consts.tile([128, NCH, H], F32, tag="xres")
    nc.sync.dma_start(
        out=xres_sb[:],
        in_=xres_d.rearrange("(nch p) h -> p nch h", p=128))

    ident_bf = consts.tile([128, 128], BF, tag="idbf")
    make_identity(nc, ident_bf[:])
    ident_f32 = consts.tile([128, 128], F32, tag="idf32")
    make_identity(nc, ident_f32[:])
    eps_sb = consts.tile([128, 1], F32, tag="eps")
    nc.vector.memset(eps_sb[:], EPS)
    zero_sb = consts.tile([128, 1], F32, tag="zero")
    nc.vector.memset(zero_sb[:], 0.0)

    # persistent intermediates
    fused_sb = consts.tile([128, S, CCH, N], BF, tag="fused")
    lowT_sb = consts.tile([BOT, N], BF, tag="lowT")
    h_all = consts.tile([128, NCH, H], F32, tag="h_all")
    mv1 = consts.tile([128, NCH, 2], F32, tag="mv1")
    rstd1 = consts.tile([128, NCH], F32, tag="rstd1")
    hT_sb = consts.tile([128, CCH, N], BF, tag="hT")
    wh_all = consts.tile([128, NCH, HEADS * (D + 1)], BF, tag="wh")
    nc.vector.memset(
        wh_all[:].rearrange("p j (h x) -> p j h x", x=D + 1)[:, :, :, D], 1.0)
    sd_sb = consts.tile([128, NCH, 2 * HEADS], F32, tag="sd")
    srcb_sb = consts.tile([128, HEADS, N], BF, tag="srcb")
    hp_all = consts.tile([128, NCH, H], F32, tag="hp")
    mv2 = consts.tile([128, NCH, 2], F32, tag="mv2")
    rstd2 = consts.tile([128, NCH], F32, tag="rstd2")

    # ---------------- phase A: conv + silu ----------------
    ctxA = ExitStack()
    convp = ctxA.enter_context(tc.tile_pool(name="convp", bufs=3, space="PSUM"))
    lowp = ctxA.enter_context(tc.tile_pool(name="lowp", bufs=2, space="PSUM"))
    for cout in range(CCH):
        for nch in range(2):           # halves of N, 512 wide
            for i in range(S):
                ps = convp.tile([128, 512], F32, tag="conv")
                dil = 2 ** i
                first = True
                for c in range(CCH):
                    for k in range(K):
                        sh = (k - 1) * dil
                        t = (i * K + k) * CCH + c
                        nc.tensor.matmul(
                            ps[:],
                            lhsT=wt_sb[:, t, cout * 128:(cout + 1) * 128],
                            rhs=xpad[:, c, 8 + sh + nch * 512:
                                     8 + sh + nch * 512 + 512],
                            start=first, stop=(c == CCH - 1 and k == K - 1))
                        first = False
                dst = fused_sb[:, i, cout, nch * 512:nch * 512 + 512]
                bias_ap = bconv_sb[:, i * CCH + cout:i * CCH + cout + 1]
                if os.environ.get("BASS_SIM_COMPAT", "0") == "1":
                    # CoreSim has no Silu: sigmoid + fused (ps+b)*sig on DVE
                    sg = work.tile([128, 512], F32, tag="sg")
                    nc.scalar.activation(
                        out=sg[:], in_=ps[:],
                        func=mybir.ActivationFunctionType.Sigmoid,
                        bias=bias_ap, scale=1.0)
                    nc.vector.scalar_tensor_tensor(
                        out=dst, in0=ps[:], scalar=bias_ap, in1=sg[:],
                        op0=mybir.AluOpType.add, op1=mybir.AluOpType.mult)
                else:
                    nc.scalar.activation(
                        out=dst, in_=ps[:],
                        func=mybir.ActivationFunctionType.Silu,
                        bias=bias_ap, scale=1.0)

    # ---------------- phase A2: lowT = sum_i (a_i W_low)^T @ silu_i --------
    for nch in range(2):
        lps = lowp.tile([BOT, 512], F32, tag="low")
        first = True
        for i in range(S):
            for c in range(CCH):
                nc.tensor.matmul(
                    lps[:],
                    lhsT=wlow_sb[:, i * CCH + c, :],
                    rhs=fused_sb[:, i, c, nch * 512:nch * 512 + 512],
                    start=first, stop=(i == S - 1 and c == CCH - 1))
                first = False
        nc.vector.tensor_copy(out=lowT_sb[:, nch * 512:nch * 512 + 512],
                              in_=lps[:])
    ctxA.close()

    # ---------------- phase B: high + residual + ln1 + transpose ----------
    ctxB = ExitStack()
    psA = ctxB.enter_context(tc.tile_pool(name="psB", bufs=2, space="PSUM"))
    psTr = ctxB.enter_context(tc.tile_pool(name="psTrB", bufs=3, space="PSUM"))
    for q in range(NCH):
        hps = psA.tile([128, H], F32, tag="high")
        nc.tensor.matmul(hps[:], lhsT=lowT_sb[:, q * 128:(q + 1) * 128],
                         rhs=whigh_sb[:], start=True, stop=True)
        nc.vector.tensor_add(h_all[:, q, :], hps[:], xres_sb[:, q, :])
        st = statp.tile([128, 6], F32, tag="bn1")
        nc.vector.bn_stats(out=st[:], in_=h_all[:, q, :])
        nc.vector.bn_aggr(out=mv1[:, q, :], in_=st[:])

    # rstd1 = exp(-0.5 * ln(var + eps))  (one table set: natural_log+exp)
    nc.scalar.activation(out=rstd1[:], in_=mv1[:, :, 1],
                         func=mybir.ActivationFunctionType.Ln, bias=eps_sb[:],
                         scale=1.0)
    nc.scalar.activation(out=rstd1[:], in_=rstd1[:],
                         func=mybir.ActivationFunctionType.Exp, bias=zero_sb[:],
                         scale=-0.5)

    for q in range(NCH):
        hn = work.tile([128, H], BF, tag="hn")
        nc.vector.tensor_scalar(
            out=hn[:], in0=h_all[:, q, :],
            scalar1=mv1[:, q, 0:1], scalar2=rstd1[:, q:q + 1],
            op0=mybir.AluOpType.subtract, op1=mybir.AluOpType.mult)
        for c in range(CCH):
            tp = psTr.tile([128, 128], BF, tag="trh")
            nc.tensor.transpose(out=tp[:], in_=hn[:, c * 128:(c + 1) * 128],
                                identity=ident_bf[:])
            nc.vector.tensor_copy(out=hT_sb[:, c, q * 128:(q + 1) * 128],
                                  in_=tp[:])

    ctxB.close()
    # ---------------- phase C: GAT projections ----------------
    ctxC = ExitStack()
    psA = ctxC.enter_context(tc.tile_pool(name="psC", bufs=2, space="PSUM"))
    psTr = ctxC.enter_context(tc.tile_pool(name="psTrC", bufs=2, space="PSUM"))
    for j in range(NCH):
        gps = psA.tile([128, H + 2 * HEADS], F32, tag="gat")
        for c in range(CCH):
            nc.tensor.matmul(gps[:], lhsT=hT_sb[:, c, j * 128:(j + 1) * 128],
                             rhs=g_sb[:, c, :], start=(c == 0),
                             stop=(c == CCH - 1))
        whj = wh_all[:, j, :].rearrange("p (h x) -> p h x", x=D + 1)
        nc.scalar.copy(
            out=whj[:, :, 0:D],
            in_=gps[:, 0:H].rearrange("p (h x) -> p h x", x=D))
        nc.vector.tensor_copy(out=sd_sb[:, j, :], in_=gps[:, H:H + 2 * HEADS])

    # src_bcast[h][p, q] = src_h[q] for all p, via replicated-column matmul
    for h in range(HEADS):
        for half in range(2):
            sps = psTr.tile([128, 512], F32, tag="sbc")
            for c in range(CCH):
                nc.tensor.matmul(
                    sps[:], lhsT=wsr_sb[:, h, c, :],
                    rhs=hT_sb[:, c, half * 512:half * 512 + 512],
                    start=(c == 0), stop=(c == CCH - 1))
            nc.scalar.copy(out=srcb_sb[:, h, half * 512:half * 512 + 512],
                           in_=sps[:])

    ctxC.close()
    # ---------------- phase D: attention ----------------
    ctxD = ExitStack()
    attp = ctxD.enter_context(tc.tile_pool(name="attp", bufs=4, space="PSUM"))
    psTr = ctxD.enter_context(tc.tile_pool(name="psTrD", bufs=3, space="PSUM"))
    for h in range(HEADS):
        ptl = ptlp.tile([128, NCH, N], BF, tag="ptl")
        pt = ptp.tile([128, NCH, N], BF, tag="pt")
        for j in range(NCH):
            # ptl = leaky(srcb[q] + dst[m] + mask) in one DVE pass
            nc.vector._custom_dve(
                LEAKY_OP, out=ptl[:, j, :], in0=srcb_sb[:, h, :],
                in1=mask_sb[:, j, :],
                s0=sd_sb[:, j, HEADS + h:HEADS + h + 1], imm2=LEAKY_SLOPE)
        for half in range(2):
            nc.scalar.activation(
                out=pt[:, half * 4:half * 4 + 4, :],
                in_=ptl[:, half * 4:half * 4 + 4, :],
                func=mybir.ActivationFunctionType.Exp,
                bias=zero_sb[:], scale=1.0)

        hp0 = attp.tile([D + 1, 512], F32, tag="hpT")
        hp1 = attp.tile([D + 1, 512], F32, tag="hpT")
        for j in range(NCH):
            for half, hps_ in ((0, hp0), (1, hp1)):
                nc.tensor.matmul(
                    hps_[:],
                    lhsT=wh_all[:, j, h * (D + 1):(h + 1) * (D + 1)],
                    rhs=pt[:, j, half * 512:half * 512 + 512],
                    start=(j == 0), stop=(j == NCH - 1))
        hpt = work.tile([D + 1, N], F32, tag="hpt")
        nc.scalar.copy(out=hpt[:, 0:512], in_=hp0[:])
        nc.scalar.copy(out=hpt[:, 512:N], in_=hp1[:])
        for q in range(NCH):
            tq = psTr.tile([128, D + 1], F32, tag="trq")
            nc.tensor.transpose(out=tq[:], in_=hpt[:, q * 128:(q + 1) * 128],
                                identity=ident_f32[0:D + 1, 0:D + 1])
            rd = statp.tile([128, 1], F32, tag="rd")
            nc.vector.reciprocal(out=rd[:], in_=tq[:, D:D + 1])
            nc.vector.tensor_scalar_mul(
                out=hp_all[:, q, h * D:(h + 1) * D],
                in0=tq[:, 0:D], scalar1=rd[:])

    ctxD.close()
    # ---------------- phase E: ln2 + out ----------------
    for q in range(NCH):
        st = statp.tile([128, 6], F32, tag="bn2")
        nc.vector.bn_stats(out=st[:], in_=hp_all[:, q, :])
        nc.vector.bn_aggr(out=mv2[:, q, :], in_=st[:])
    nc.scalar.activation(out=rstd2[:], in_=mv2[:, :, 1],
                         func=mybir.ActivationFunctionType.Ln, bias=eps_sb[:],
                         scale=1.0)
    nc.scalar.activation(out=rstd2[:], in_=rstd2[:],
                         func=mybir.ActivationFunctionType.Exp, bias=zero_sb[:],
                         scale=-0.5)
    for q in range(NCH):
        ot = outp.tile([128, H], F32, tag="out")
        nc.vector.tensor_scalar(
            out=ot[:], in0=hp_all[:, q, :],
            scalar1=mv2[:, q, 0:1], scalar2=rstd2[:, q:q + 1],
            op0=mybir.AluOpType.subtract, op1=mybir.AluOpType.mult)
        nc.sync.dma_start(out=out_d[q * 128:(q + 1) * 128, :], in_=ot[:])


def _prep(inputs):
    """Host-side parameter folding. Returns per-core input maps."""
    bf16 = ml_dtypes.bfloat16
    f = lambda a: np.ascontiguousarray(np.asarray(a, np.float32))

    x = f(inputs["x"])
    adj = np.asarray(inputs["adj"])
    conv_w = f(inputs["conv_w"]); conv_b = f(inputs["conv_b"])
    bn_g = f(inputs["bn_g"]); bn_b = f(inputs["bn_b"])
    fw = f(inputs["fusion_weight"])
    W_low = f(inputs["W_low"]); b_low = f(inputs["b_low"])
    W_high = f(inputs["W_high"]); b_high = f(inputs["b_high"])
    ln1_g = f(inputs["ln1_g"]); ln1_b = f(inputs["ln1_b"])
    gat_W = f(inputs["gat_W"])
    a_src = f(inputs["a_src"]); a_dst = f(inputs["a_dst"])
    ln2_g = f(inputs["ln2_g"]); ln2_b = f(inputs["ln2_b"])

    trivial = dict(
        b_low=np.allclose(b_low, 0), b_high=np.allclose(b_high, 0),
        ln1=np.allclose(ln1_g, 1) and np.allclose(ln1_b, 0),
        ln2=np.allclose(ln2_g, 1) and np.allclose(ln2_b, 0))
    if not all(trivial.values()):
        raise NotImplementedError(f"non-trivial affine params: {trivial}")

    alpha = np.exp(fw - fw.max()); alpha /= alpha.sum()
    gprime = bn_g / np.float32(np.sqrt(1.0 + EPS))          # [S,H]
    bconv = conv_b * gprime + bn_b                           # [S,H]
    # Wt[i,k,cin,cout] = conv_w[i,cout,cin,k]*gprime[i,cout]
    Wt = np.transpose(conv_w, (0, 3, 2, 1)) * gprime[:, None, None, :]
    # [S,K,cin,H] -> [S,K,CCH,128,H] -> [S*K*CCH,128,H]
    Wt = Wt.reshape(S, K, CCH, 128, H).reshape(S * K * CCH, 128, H)
    # bconv laid out [128, S*CCH]: column i*CCH+c holds channels c*128..c*128+127
    bconv_t = bconv.reshape(S, CCH, 128).transpose(2, 0, 1).reshape(128, S * CCH)

    WlowA = (alpha[:, None, None] * W_low[None]).reshape(S, CCH, 128, BOT)
    WlowA = WlowA.reshape(S * CCH, 128, BOT)

    G = np.zeros((H, H + 2 * HEADS), np.float32)
    for h in range(HEADS):
        G[:, h * D:(h + 1) * D] = gat_W[h]
        G[:, H + h] = gat_W[h] @ a_src[h]
        G[:, H + HEADS + h] = gat_W[h] @ a_dst[h]
    Gr = G.reshape(CCH, 128, H + 2 * HEADS)

    maskT = np.where(adj.T > 0, np.float32(0.0), np.float32(NEG))
    maskTr = maskT.reshape(NCH, 128, N)

    # wsrcrep[h, c, :, j] = (gat_W[h] @ a_src[h])[c*128 + :]  (all 128 cols equal)
    wsrc = np.stack([gat_W[h] @ a_src[h] for h in range(HEADS)])  # [HEADS, H]
    wsrcrep = np.repeat(
        wsrc.reshape(HEADS, CCH, 128, 1), 128, axis=3).astype(np.float32)

    shared = {
        "wt": Wt.astype(bf16),
        "bconv": np.ascontiguousarray(bconv_t),
        "wlow": WlowA.astype(bf16),
        "whigh": W_high.astype(bf16),
        "gmat": Gr.astype(bf16),
        "maskT": maskTr.astype(bf16),
        "wsrcrep": wsrcrep.astype(bf16),
    }
    in_maps = []
    for b in range(B):
        xt = np.ascontiguousarray(x[b].T)                    # [H, N]
        m = dict(shared)
        m["xt"] = xt.reshape(CCH, 128, N).astype(bf16)
        m["xres"] = np.ascontiguousarray(x[b])
        in_maps.append(m)
    return in_maps, trivial


def kernel(**inputs) -> np.ndarray:
    in_maps, trivial = _prep(inputs)
    key = "k"
    if key not in _CACHED:
        _CACHED[key] = _build(trivial)
    nc = _CACHED[key]
    res = run_bass_kernel_spmd(nc, in_maps, list(range(B)))
    out = np.stack([res.results[i]["out"] for i in range(B)], axis=0)
    return out.astype(np.float32)


if __name__ == "__main__":
    import reference
    inputs = {k: np.asarray(v) for k, v in reference.setup_inputs().items()}
    got = kernel(**inputs)
    print("kernel output", got.shape, got.dtype)
